# revision 1
# baseline (speedup 1.0000x reference)
"""Trainium2 Bass kernel for SimCLR-style contrastive (NT-Xent) loss.

Reference computation:
    z = concat(emb_i, emb_j)            # [8192, 256]
    z = z / ||z||_row
    sim = (z @ z.T) / 0.5               # [8192, 8192]
    sim[i, i] = -inf
    loss = mean_i( logsumexp_j(sim[i, :]) - sim[i, label_i] )
    label_i = (i + 4096) % 8192

Distribution: data-parallel over the 8192 rows, 1024 rows per core. Every
core receives the full z, but ROTATED by its row offset (np.roll on host =
pure resharding), so the SPMD program is identical on all cores: "my" rows
are always rows 0..1023 and their positive pairs are always rows 4096..5119.
Row sums of exp(sim) are invariant to the column rotation.

Per-core kernel:
  phase 1: load z (fp32), per-row sumsq on VectorE (scalar_tensor_tensor
           with accum_out), rinv = exp(-0.5*ln(ss)) on ScalarE (stays in
           the single natural_log_exp table set), normalize+cast to bf16
           on VectorE, then ONE DMA-xbar transpose per 2048-col group into
           an interleaved znT layout [128, (tile, dhalf, col)] — batched
           transposes are ~10x cheaper than per-128x128-block ones.
  phase 2: for each of 8 local row-tiles x 4 column groups: TensorE computes
           a [128, 2048] block of G = znT.T @ znT into PSUM (K=256 as two
           accumulated matmuls), ScalarE computes exp(2G - 2) with fused
           per-row accumulation (accum_out). 2.0 == max possible sim, so
           this is a numerically safe softmax shift with no row-max pass.
  phase 3: S = sum of partials; masked sum Sm = S - exp(self_sim - 2);
           per-row loss = 2 + ln(Sm) - pair_sim, where self_sim/pair_sim
           come from cheap 256-length row dots on VectorE.
Host: loss = sum(all per-core per-row losses) / 8192.
"""

import os
import sys
from contextlib import ExitStack

import numpy as np

for _p in ("/opt/trn_rl_repo",):
    if os.path.isdir(_p) and _p not in sys.path:
        sys.path.insert(0, _p)

import concourse.bacc as bacc
import concourse.tile as tile
from concourse import mybir
from concourse.bass_utils import run_bass_kernel_spmd

F32 = mybir.dt.float32
BF16 = mybir.dt.bfloat16
AF = mybir.ActivationFunctionType
ALU = mybir.AluOpType

N, D = 8192, 256          # 2B rows, feature dim
NCORES = 8
ROWS = N // NCORES        # 1024 rows owned per core
NT = N // 128             # 64 row-tiles of 128 rows
NG = NT // 8              # 8 load groups (8 tiles = 1 MiB per DMA)
RT = ROWS // 128          # 8 local row-tiles per core
CG = 4                    # column groups of 2048 (4 PSUM banks) per row-tile

_ACT_SET = "natural_log_exp_and_others"   # contains exp, ln, square, copy


def _patch_act_tables():
    """Restrict the ACT table-set chooser to the one set containing every
    function this kernel uses. Without this, bacc's chooser alternates
    between exp/ln sets and inserts ~18 ACT_TABLE_LOADs (~23us). Entries
    keep their positions so act_func_set_id indices stay valid."""
    if getattr(bacc, "_act_tables_patched", False):
        return
    orig = bacc.get_activation_tables

    def restricted(arch):
        full = dict(orig(arch))
        return {
            name: (fns if name == _ACT_SET else set())
            for name, fns in full.items()
        }

    bacc.get_activation_tables = restricted
    bacc._act_tables_patched = True


def _build_kernel(ctx, tc, z, out):
    nc = tc.nc
    zr = z.rearrange("(a p) d -> p a d", p=128)  # [128, 64, 256] view of DRAM

    staging = ctx.enter_context(tc.tile_pool(name="staging", bufs=4))
    znpool = ctx.enter_context(tc.tile_pool(name="znpool", bufs=3))
    persist = ctx.enter_context(tc.tile_pool(name="persist", bufs=1))
    epool = ctx.enter_context(tc.tile_pool(name="epool", bufs=2))
    ppool = ctx.enter_context(tc.tile_pool(name="ppool", bufs=2, space="PSUM"))

    # persistent tensors.
    # znTi is the transposed z in an interleaved layout: column index is
    # (tile t, dhalf a, row-within-tile c) -> t*256 + a*128 + c, i.e.
    # znTi[d, t*256 + a*128 + c] = zn[t*128 + c, a*128 + d].
    znTi = persist.tile([128, 2 * N], BF16)
    znself = persist.tile([128, 8 * D], BF16)  # normalized rows 0..1023 (row-major)
    znpair = persist.tile([128, 8 * D], BF16)  # normalized rows 4096..5119
    ss = persist.tile([128, NT], F32)          # per-row sum of squares
    lss = persist.tile([128, NT], F32)
    rinv = persist.tile([128, NT], F32)        # 1/||z_row||
    sqjunk = persist.tile([128, D], F32)       # unused sumsq elementwise output
    ttrjunk = persist.tile([128, D], BF16)     # unused dot elementwise output
    selfs = persist.tile([128, RT], F32)       # sim[g, g] per local row
    numers = persist.tile([128, RT], F32)      # sim[g, g+4096] per local row
    sparts = persist.tile([128, RT * CG], F32)  # partial exp-row-sums
    negtwo = persist.tile([128, 1], F32)       # bias constant for exp(x - 2)
    nc.vector.memset(negtwo[:], -2.0)

    # ---- Phase 1: load + normalize + transpose ----
    # Software-pipelined emission: sumsq/ln/exp of group k is emitted BEFORE
    # the scales of group k-1, so the in-order VectorE queue never stalls
    # waiting for ScalarE to produce rinv of the group it is about to scale.
    def emit_head(k):
        st = staging.tile([128, 8, D], F32, tag="st", name="st")
        # 4 sub-loads per group: subtile deps let each pair of sumsq ops
        # start as soon as its 256KB quarter lands.
        for q in range(4):
            nc.sync.dma_start(
                st[:, q * 2:(q + 1) * 2, :],
                zr[:, k * 8 + q * 2:k * 8 + (q + 1) * 2, :],
            )
        for t8 in range(8):
            t = k * 8 + t8
            nc.vector.scalar_tensor_tensor(
                out=sqjunk[:], in0=st[:, t8, :], scalar=1.0, in1=st[:, t8, :],
                op0=ALU.mult, op1=ALU.mult, accum_out=ss[:, t:t + 1],
            )
        gsl = slice(k * 8, (k + 1) * 8)
        nc.scalar.activation(lss[:, gsl], ss[:, gsl], AF.Ln)
        nc.scalar.activation(rinv[:, gsl], lss[:, gsl], AF.Exp, scale=-0.5)
        return st

    def emit_tail(k, st):
        if k == 0:
            zng = znself
        elif k == 4:
            zng = znpair
        else:
            zng = znpool.tile([128, 8 * D], BF16, tag="zng", name="zng")
        for t8 in range(8):
            t = k * 8 + t8
            nc.vector.tensor_scalar_mul(
                zng[:, t8 * D:(t8 + 1) * D], st[:, t8, :], rinv[:, t:t + 1]
            )
        # one batched xbar transpose for the whole group: out[d, u, c] =
        # zng[c, u*128 + d] for u = t8*2 + a -- exactly the interleaved
        # layout slice znTi[:, k*2048 : (k+1)*2048].
        o3 = znTi[:, k * 2048:(k + 1) * 2048].rearrange(
            "p (u c) -> p u c", c=128
        )
        nc.sync.dma_start_transpose(o3, zng[:])

    prev = None
    for k in range(NG):
        st = emit_head(k)
        if prev is not None:
            emit_tail(k - 1, prev)
        prev = st
    emit_tail(NG - 1, prev)

    # ---- self / positive-pair similarities from row-major normalized rows ----
    # (emitted after all scales; only needed by phase 3)
    for r in range(RT):
        rsl = slice(r * D, (r + 1) * D)
        nc.vector.scalar_tensor_tensor(
            out=ttrjunk[:], in0=znself[:, rsl], scalar=2.0, in1=znself[:, rsl],
            op0=ALU.mult, op1=ALU.mult, accum_out=selfs[:, r:r + 1],
        )
        nc.vector.scalar_tensor_tensor(
            out=ttrjunk[:], in0=znself[:, rsl], scalar=2.0, in1=znpair[:, rsl],
            op0=ALU.mult, op1=ALU.mult, accum_out=numers[:, r:r + 1],
        )

    # ---- Phase 2: Gram blocks + fused exp row-sums ----
    # Group-major order: column group g only needs phase-1 transpose groups
    # 2g and 2g+1, so TensorE/ScalarE start consuming while VectorE is
    # still normalizing later groups.
    znTr = znTi.rearrange("p (t a c) -> p t a c", a=2, c=128)  # [128,64,2,128]
    for g in range(CG):
        for r in range(RT):
            lhs0 = znTr[:, r, 0, :]
            lhs1 = znTr[:, r, 1, :]
            ps = ppool.tile([128, 2048], F32, tag="ps", name="ps")
            for a, lhs in ((0, lhs0), (1, lhs1)):
                for s in range(4):
                    tsl = slice((g * 4 + s) * 4, (g * 4 + s + 1) * 4)
                    nc.tensor.matmul(
                        ps[:, s * 512:(s + 1) * 512], lhs,
                        znTr[:, tsl, a, :],
                        start=(a == 0), stop=(a == 1),
                    )
            e = epool.tile([128, 2048], BF16, tag="e", name="e")
            nc.scalar.activation(
                e[:], ps[:], AF.Exp, bias=negtwo[:, 0:1], scale=2.0,
                accum_out=sparts[:, r * CG + g:r * CG + g + 1],
            )

    # ---- Phase 3: finals ----
    S = persist.tile([128, RT], F32)
    nc.vector.tensor_reduce(
        S[:], sparts.rearrange("p (r g) -> p r g", g=CG),
        axis=mybir.AxisListType.X, op=ALU.add,
    )
    ediag = persist.tile([128, RT], F32)
    nc.scalar.activation(ediag[:], selfs[:], AF.Exp, bias=negtwo[:, 0:1])
    sm = persist.tile([128, RT], F32)
    nc.vector.tensor_sub(sm[:], S[:], ediag[:])
    lsm = persist.tile([128, RT], F32)
    nc.scalar.activation(lsm[:], sm[:], AF.Ln)
    lossp = persist.tile([128, RT], F32)
    nc.vector.scalar_tensor_tensor(
        out=lossp[:], in0=lsm[:], scalar=2.0, in1=numers[:],
        op0=ALU.add, op1=ALU.subtract,
    )
    nc.sync.dma_start(out[:], lossp[:])


_CACHE = {}


def get_nc():
    if "nc" not in _CACHE:
        _patch_act_tables()
        nc = bacc.Bacc(
            "TRN2", target_bir_lowering=False, debug=False, num_devices=NCORES
        )
        z = nc.dram_tensor("z", [N, D], F32, kind="ExternalInput").ap()
        out = nc.dram_tensor("out_partial", [128, RT], F32, kind="ExternalOutput").ap()
        with tile.TileContext(nc) as tc:
            with ExitStack() as ctx:
                _build_kernel(ctx, tc, z, out)
        nc.compile()
        _CACHE["nc"] = nc
    return _CACHE["nc"]


def make_in_maps(embeddings_i, embeddings_j):
    ei = np.ascontiguousarray(np.asarray(embeddings_i), dtype=np.float32)
    ej = np.ascontiguousarray(np.asarray(embeddings_j), dtype=np.float32)
    z = np.concatenate([ei, ej], axis=0)
    return [
        {"z": np.ascontiguousarray(np.roll(z, -ROWS * c, axis=0))}
        for c in range(NCORES)
    ]


def reduce_results(results):
    total = 0.0
    for r in results:
        total += r["out_partial"].astype(np.float64).sum()
    return np.float32(total / N)


def run(embeddings_i, embeddings_j, **spmd_kwargs):
    nc = get_nc()
    in_maps = make_in_maps(embeddings_i, embeddings_j)
    res = run_bass_kernel_spmd(nc, in_maps, list(range(NCORES)), **spmd_kwargs)
    return reduce_results(res.results), res


def kernel(embeddings_i, embeddings_j):
    loss, _ = run(embeddings_i, embeddings_j)
    return loss

